# revision 15
# baseline (speedup 1.0000x reference)
"""RNN-T joint network kernel for Trainium2 (8 NeuronCores, SPMD).

out[b,t,u,v] = (enc[b,t] @ W_enc.T)[v] + (dec[b,u] @ W_dec.T)[v]

Shapes: enc (4,512,512), dec (4,128,512), W (1024,1024) -> out (4,512,128,1024).

v2 strategy (fp16 output, rel tolerance is 2e-2 so fp16 store is free accuracy-wise):
  - shard T across the 8 cores (64 rows each). Inputs and output in fp16:
    the per-core HBM write drops from 134 MB to 67 MB -> ~187us roofline/core.
  - host pre-transposes inputs to contraction-major fp16; projections are
    fp16 matmuls accumulated in fp32 PSUM, copied to SBUF as fp16 by ACT.
  - broadcast-add as big DVE tensor_tensor fp16 ops [128, 8192] in 2x_1p
    mode (4.33us each): the e-term is pre-replicated only 8x into a small
    e_small[v,(b,t,j8)] tile so BOTH operands keep innermost stride +1
    (access pattern (t, rep16, j8)); the d-term broadcasts along t (outer
    stride 0). DVE total ~150us, under the DMA floor.
  - output written in device layout (B, VT, 128, T_loc*U) fp16, one 2 MB
    DMA per (b, m); host upconverts + transposes when gathering.
"""

import sys

if "/opt/trn_rl_repo" not in sys.path:
    sys.path.insert(0, "/opt/trn_rl_repo")

import numpy as np

# Problem shape (hardcoded per contract)
B, T, U, D, V = 4, 512, 128, 512, 1024
N_CORES = 8
P = 128

T_LOC = T // N_CORES          # 64 t-rows per core
TOK = B * T_LOC               # 256 (b,t) rows per core
KT = D // P                   # 4 contraction tiles
VT = V // P                   # 8 v tiles
BU = B * U                    # 512
CHUNK = T_LOC * U             # 8192 free elems per (b, m) chunk
J = 8                         # e_small replication run
R = U // J                    # 16

_CACHE: dict = {}


def _emit(tc, aps, mybir):
    from contextlib import ExitStack

    nc = tc.nc
    f16 = mybir.dt.float16
    f32 = mybir.dt.float32
    encT, decT, out = aps["encT"], aps["decT"], aps["out"]

    with ExitStack() as ctx:
        const = ctx.enter_context(tc.tile_pool(name="const", bufs=1))
        psum = ctx.enter_context(tc.tile_pool(name="psum", bufs=4, space="PSUM"))
        esm = ctx.enter_context(tc.tile_pool(name="esm", bufs=3))
        stage = ctx.enter_context(tc.tile_pool(name="stage", bufs=8))

        # --- input loads, critical-path first ---
        def load(src, tag):
            """One flat line-rate DMA on the ACT HWDGE queue (keeps the SP
            queue free for the output stream). Host pre-tiles to [P, kt*w]."""
            t = const.tile([P, src.shape[1]], f16, tag=tag)
            nc.scalar.dma_start(out=t[:], in_=src)
            return t

        wenc_m0 = load(aps["wenc0"], "wenc0")      # [P, 4*128]
        enc_t = load(encT, "enc")                  # [P, 4*256]
        wdec_m0 = load(aps["wdec0"], "wdec0")      # [P, 4*128]
        dec_t = load(decT, "dec")                  # [P, 4*512]
        # remaining weights, m-major: one 128 KB flat DMA per (m, enc/dec) so
        # m=1's projections unblock as soon as its slice lands
        MW = KT * P  # 512 cols per m-block
        wenc_r = const.tile([P, (VT - 1) * MW], f16, tag="wencr")
        wdec_r = const.tile([P, (VT - 1) * MW], f16, tag="wdecr")
        for mm in range(VT - 1):
            nc.scalar.dma_start(
                out=wenc_r[:, mm * MW : (mm + 1) * MW],
                in_=aps["wencr"][:, mm * MW : (mm + 1) * MW],
            )
            nc.scalar.dma_start(
                out=wdec_r[:, mm * MW : (mm + 1) * MW],
                in_=aps["wdecr"][:, mm * MW : (mm + 1) * MW],
            )

        def project(lhs, lhs_w, lhs_lo, rhs, rhs_w, n, tag, n0=None):
            """psum[P, n] = sum_k lhs[:, k*lhs_w+lhs_lo : +128].T @ rhs[:, k*rhs_w : +n];
            ACT-copy to SBUF fp16. If n0 is set, the first n0 columns are computed,
            copied, and usable before the rest (shorter critical path for b=0)."""
            ps = psum.tile([P, n], f32, tag="ps" + tag[0])
            sb = const.tile([P, n], f16, tag=tag)
            splits = [(0, n0), (n0, n)] if n0 else [(0, n)]
            for lo, hi in splits:
                for k in range(KT):
                    nc.tensor.matmul(
                        ps[:, lo:hi],
                        lhsT=lhs[:, k * lhs_w + lhs_lo : k * lhs_w + lhs_lo + P],
                        rhs=rhs[:, k * rhs_w + lo : k * rhs_w + hi],
                        start=(k == 0),
                        stop=(k == KT - 1),
                    )
                nc.scalar.activation(
                    sb[:, lo:hi], ps[:, lo:hi], mybir.ActivationFunctionType.Copy
                )
            return sb

        def emit_chunk(es, dproj, b, m, n_pieces, eproj32=None):
            """TT + DMA for chunk (b, m), split into n_pieces along t.
            If eproj32 is given, compute on ACT (per-t Identity+bias) instead of
            DVE — used during the ramp while ACT is otherwise idle."""
            tw = T_LOC // n_pieces
            for i in range(n_pieces):
                S = stage.tile([P, tw * U], f16, tag="stage")
                t0 = i * tw
                if eproj32 is not None:
                    for tt in range(tw):
                        nc.scalar.activation(
                            S[:, tt * U : (tt + 1) * U],
                            dproj[:, b * U : (b + 1) * U],
                            mybir.ActivationFunctionType.Identity,
                            bias=eproj32[:, t0 + tt : t0 + tt + 1],
                        )
                else:
                    e_in = (
                        es[:, (b * T_LOC + t0) * J : (b * T_LOC + t0 + tw) * J]
                        .rearrange("p (t j) -> p t j", j=J)[:, :, None, :]
                        .to_broadcast((P, tw, R, J))
                    )
                    d_in = (
                        dproj[:, b * U : (b + 1) * U]
                        .rearrange("p (r j) -> p r j", j=J)[:, None, :, :]
                        .to_broadcast((P, tw, R, J))
                    )
                    nc.vector.tensor_tensor(
                        S[:].rearrange("p (t r j) -> p t r j", r=R, j=J),
                        e_in,
                        d_in,
                        mybir.AluOpType.add,
                    )
                nc.sync.dma_start(
                    out=out[b, m, :, t0 * U : (t0 + tw) * U], in_=S[:]
                )

        for m in range(VT):
            if m == 0:
                we, we_w, we_lo = wenc_m0, P, 0
                wd, wd_w, wd_lo = wdec_m0, P, 0
            else:
                we, we_w, we_lo = wenc_r, P, (m - 1) * MW
                wd, wd_w, wd_lo = wdec_r, P, (m - 1) * MW

            # [P,(b,t)] / [P,(b,u)]; for m=0, b=0's columns come out first
            eproj = project(we, we_w, we_lo, enc_t, TOK, TOK, f"eproj{m}",
                            n0=T_LOC if m == 0 else None)
            dproj = project(wd, wd_w, wd_lo, dec_t, BU, BU, f"dproj{m}",
                            n0=U if m == 0 else None)

            # e_small_m [P, (b, t, j)]: e replicated J times along j (DVE 2x copy)
            es = esm.tile([P, B * T_LOC * J], f16, tag="esmall")
            for blo, bhi in ([(0, 1), (1, B)] if m == 0 else [(0, B)]):
                nc.vector.tensor_copy(
                    out=es[:, blo * T_LOC * J : bhi * T_LOC * J].rearrange(
                        "p (b t j) -> p b t j", t=T_LOC, j=J
                    ),
                    in_=eproj[:, blo * T_LOC : bhi * T_LOC]
                    .rearrange("p (b t) -> p b t", t=T_LOC)[:, :, :, None]
                    .to_broadcast((P, bhi - blo, T_LOC, J)),
                )

            # stage[v, (t, r, j)] = e_small[v, (t, j)] bcast over r
            #                      + dproj[v, (r, j)] bcast over t  (all stride+1 innermost)
            # Early m's chunks are split into smaller TT+DMA pieces so the
            # output stream saturates the DMA queues during the ramp; m=0's
            # b=3 chunk runs on the otherwise-idle ACT engine.
            if m == 0:
                ep32 = const.tile([P, T_LOC], f32, tag="ep32")
                nc.scalar.activation(
                    ep32[:],
                    eproj[:, 3 * T_LOC : 4 * T_LOC],
                    mybir.ActivationFunctionType.Copy,
                )
                emit_chunk(es, dproj, 0, m, 4)
                emit_chunk(es, dproj, 3, m, 4, eproj32=ep32)
                emit_chunk(es, dproj, 1, m, 4)
                emit_chunk(es, dproj, 2, m, 4)
            else:
                n_pieces = 2 if m == 1 else 1
                for b in range(B):
                    emit_chunk(es, dproj, b, m, n_pieces)


def build_bass(num_devices=N_CORES):
    key = ("nc", num_devices)
    if key in _CACHE:
        return _CACHE[key]
    import concourse.bacc as bacc
    import concourse.tile as tile
    from concourse import mybir

    nc = bacc.Bacc(
        "TRN2",
        target_bir_lowering=False,
        debug=False,
        num_devices=num_devices,
    )
    f16 = mybir.dt.float16
    aps = {
        "encT": nc.dram_tensor("encT", [P, KT * TOK], f16, kind="ExternalInput").ap(),
        "decT": nc.dram_tensor("decT", [P, KT * BU], f16, kind="ExternalInput").ap(),
        "wenc0": nc.dram_tensor("wenc0", [P, KT * P], f16, kind="ExternalInput").ap(),
        "wencr": nc.dram_tensor(
            "wencr", [P, KT * (V - P)], f16, kind="ExternalInput"
        ).ap(),
        "wdec0": nc.dram_tensor("wdec0", [P, KT * P], f16, kind="ExternalInput").ap(),
        "wdecr": nc.dram_tensor(
            "wdecr", [P, KT * (V - P)], f16, kind="ExternalInput"
        ).ap(),
        "out": nc.dram_tensor(
            "out", [B, VT, P, CHUNK], f16, kind="ExternalOutput"
        ).ap(),
    }
    with tile.TileContext(nc) as tc:
        _emit(tc, aps, mybir)
    nc.compile()
    _CACHE[key] = nc
    return nc


def _tile_kmajor(srcT):
    """[D, w] contraction-major -> [P, KT*w] pre-tiled for a flat SBUF load."""
    w = srcT.shape[1]
    return np.ascontiguousarray(
        srcT.reshape(KT, P, w).transpose(1, 0, 2).reshape(P, KT * w)
    ).astype(np.float16)


def make_in_maps(encoder_outputs, decoder_outputs, fc_weight):
    enc = np.asarray(encoder_outputs, dtype=np.float32)
    dec = np.asarray(decoder_outputs, dtype=np.float32)
    w = np.asarray(fc_weight, dtype=np.float32)
    decT = _tile_kmajor(dec.reshape(BU, D).T)
    wencT = w[:, :D].T  # [D, V]
    wdecT = w[:, D:].T
    wenc0, wdec0 = _tile_kmajor(wencT[:, :P]), _tile_kmajor(wdecT[:, :P])
    # m-major remaining blocks: [P, 7*KT*P], block m' = cols (m'+1)*P:(m'+2)*P
    wencr = np.concatenate(
        [_tile_kmajor(wencT[:, (i + 1) * P : (i + 2) * P]) for i in range(VT - 1)],
        axis=1,
    )
    wdecr = np.concatenate(
        [_tile_kmajor(wdecT[:, (i + 1) * P : (i + 2) * P]) for i in range(VT - 1)],
        axis=1,
    )
    in_maps = []
    for c in range(N_CORES):
        enc_c = enc[:, c * T_LOC : (c + 1) * T_LOC, :].reshape(TOK, D)
        in_maps.append(
            {
                "encT": _tile_kmajor(enc_c.T),
                "decT": decT,
                "wenc0": wenc0,
                "wencr": wencr,
                "wdec0": wdec0,
                "wdecr": wdecr,
            }
        )
    return in_maps


def assemble(results):
    """results: list of per-core {"out": (B,VT,P,CHUNK) fp16} -> (B,T,U,V) fp32."""
    full = np.empty((B, T, U, V), dtype=np.float32)
    for c in range(N_CORES):
        arr = results[c]["out"].reshape(B, V, T_LOC, U)
        full[:, c * T_LOC : (c + 1) * T_LOC] = arr.transpose(0, 2, 3, 1)
    return full


def kernel(encoder_outputs, decoder_outputs, fc_weight):
    from concourse.bass_utils import run_bass_kernel_spmd

    nc = build_bass()
    in_maps = make_in_maps(encoder_outputs, decoder_outputs, fc_weight)
    res = run_bass_kernel_spmd(nc, in_maps, list(range(N_CORES)))
    return assemble(res.results)
